# revision 7
# baseline (speedup 1.0000x reference)
"""Trainium2 Bass kernel for a 16-head causal MHA block (B=4, S=2048, D=1024).

Sharding: tensor-parallel over heads — 8 cores x 2 heads each. The
reference's final reshape is a raw [B,H,S,hd]->[B,S,H*hd] view (no head
transpose), so each output row s' draws from exactly one head
(h = s'//128): the output projection is head-local and no collective is
needed. Core c computes output rows [128*2c, 128*(2c+2)) of every batch.

Device-side per core:
  qhT/khT[h]  = (x[b] @ W[h]).T        via lhsT=W-tiles, rhs=xT-tiles (f32r)
  scoresT     = khT.T @ qhT            [k_pos, q_pos], 2 heads row-packed
  ET          = exp(scoresT/8)         ACT, causal mask applied on diagonal
  attn_outT   = vh_aug.T @ ET          vh_aug has a ones column -> rowsum
  normalize   = attn_outT * (1/rowsum) broadcast via a K=1 matmul
  out rows    = sum_m A_m @ Wo[64m:64m+64]   (the raw-view projection)

All matmuls use float32r (full-rate PE path; fp32 is 4x slower).
"""

import numpy as np

B, S, D, H, HD = 4, 2048, 1024, 16, 64
NCORES = 8
HPC = H // NCORES        # heads per core = 2
SH = S // H              # output rows per head = 128
NDT = D // 128           # 8 contraction tiles for projections
NSC = S // 512           # 4 s-chunks of 512
NKT = S // 128           # 16 k-tiles
NQC = S // 512           # 4 q-chunks of 512

_CACHE = {}


def _build_nc():
    import concourse.mybir as mybir
    import concourse.tile as tile
    from concourse import bacc

    F32 = mybir.dt.float32
    F32R = mybir.dt.float32r
    EXP = mybir.ActivationFunctionType.Exp

    nc = bacc.Bacc("TRN2", target_bir_lowering=False, debug=False,
                   num_devices=NCORES)

    qT = nc.dram_tensor("qT", [B, D, S], F32R, kind="ExternalInput").ap()
    kT = nc.dram_tensor("kT", [B, D, S], F32R, kind="ExternalInput").ap()
    vT = nc.dram_tensor("vT", [B, D, S], F32R, kind="ExternalInput").ap()
    wq = nc.dram_tensor("wq", [128, D], F32R, kind="ExternalInput").ap()
    wk = nc.dram_tensor("wk", [128, D], F32R, kind="ExternalInput").ap()
    wv = nc.dram_tensor("wv", [128, D], F32R, kind="ExternalInput").ap()
    wo = nc.dram_tensor("wo", [64, H * D], F32R, kind="ExternalInput").ap()
    masks = nc.dram_tensor("masks", [128, 4 * 512], F32R, kind="ExternalInput").ap()
    ident = nc.dram_tensor("ident", [128, 128], F32R, kind="ExternalInput").ap()
    ones = nc.dram_tensor("ones", [128, 64], F32R, kind="ExternalInput").ap()
    out = nc.dram_tensor("out", [B, HPC, SH, D], F32, kind="ExternalOutput").ap()

    with tile.TileContext(nc) as tc:
        with tc.tile_pool(name="const", bufs=1) as cst, \
             tc.tile_pool(name="stage", bufs=1) as stage, \
             tc.tile_pool(name="xT", bufs=10) as pxT, \
             tc.tile_pool(name="qhT", bufs=2) as pqh, \
             tc.tile_pool(name="khT", bufs=2) as pkh, \
             tc.tile_pool(name="vhT", bufs=3) as pvh, \
             tc.tile_pool(name="vha", bufs=40) as pvha, \
             tc.tile_pool(name="et", bufs=3) as pet, \
             tc.tile_pool(name="attn", bufs=3) as patt, \
             tc.tile_pool(name="rv", bufs=2) as prv, \
             tc.tile_pool(name="osb", bufs=3) as posb, \
             tc.tile_pool(name="wo", bufs=18) as pwo, \
             tc.tile_pool(name="psp", bufs=2, space="PSUM") as psp, \
             tc.tile_pool(name="pss", bufs=2, space="PSUM") as pss, \
             tc.tile_pool(name="pso", bufs=2, space="PSUM") as pso:

            # ---- constants ------------------------------------------------
            # Weights feeding matmuls go through a DVE copy so every matmul
            # needs only one sync wait (walrus S3_LW limit).
            w_sb = {}
            for name, src in (("wq", wq), ("wk", wk), ("wv", wv)):
                raw = stage.tile([128, D], F32R, tag="wstage")
                nc.sync.dma_start(raw[:], src[:])
                cooked = cst.tile([128, D], F32R, tag=name)
                nc.vector.tensor_copy(cooked[:], raw[:])
                w_sb[name] = cooked
            ident_raw = stage.tile([128, 128], F32R, tag="identr")
            nc.sync.dma_start(ident_raw[:], ident[:])
            ident_sb = cst.tile([128, 128], F32R, tag="ident")
            nc.vector.tensor_copy(ident_sb[:], ident_raw[:])
            ones_raw = stage.tile([128, 64], F32R, tag="onesr")
            nc.sync.dma_start(ones_raw[:], ones[:])
            ones_sb = cst.tile([128, 64], F32R, tag="ones")
            nc.vector.tensor_copy(ones_sb[:], ones_raw[:])
            # masks feed DVE only (no wait limit) — direct DMA is fine
            masks_sb = cst.tile([128, 4 * 512], F32R, tag="masks")
            nc.sync.dma_start(masks_sb[:], masks[:])

            for b in range(B):
                # ---- projections ------------------------------------------
                qh_t = pqh.tile([128, S], F32R, tag="qhT")
                kh_t = pkh.tile([128, S], F32R, tag="khT")
                vh_aug = [[None] * NKT for _ in range(HPC)]
                for tname, src, w in (("q", qT, w_sb["wq"]),
                                      ("k", kT, w_sb["wk"]),
                                      ("v", vT, w_sb["wv"])):
                    for sc in range(NSC):
                        xts = []
                        for dt in range(NDT):
                            xt = pxT.tile([128, 512], F32R, tag="xT")
                            nc.sync.dma_start(
                                xt[:],
                                src[b, dt * 128:(dt + 1) * 128,
                                    sc * 512:(sc + 1) * 512])
                            xts.append(xt)
                        ps = psp.tile([128, 512], F32, tag="psp")
                        for dt in range(NDT):
                            nc.tensor.matmul(
                                ps[:], w[:, dt * 128:(dt + 1) * 128], xts[dt][:],
                                start=(dt == 0), stop=(dt == NDT - 1))
                        if tname == "q":
                            nc.vector.tensor_copy(
                                qh_t[:, sc * 512:(sc + 1) * 512], ps[:])
                        elif tname == "k":
                            nc.vector.tensor_copy(
                                kh_t[:, sc * 512:(sc + 1) * 512], ps[:])
                        else:
                            vv = pvh.tile([128, 512], F32R, tag="vhT")
                            nc.vector.tensor_copy(vv[:], ps[:])
                            for j in range(4):
                                pt = psp.tile([128, 128], F32R, tag="psp")
                                nc.tensor.transpose(
                                    pt[:], vv[:, j * 128:(j + 1) * 128],
                                    ident_sb[:])
                                st = sc * 4 + j
                                for h in range(HPC):
                                    va = pvha.tile([128, 65], F32R, tag="vha")
                                    nc.vector.tensor_copy(
                                        va[:, 0:64],
                                        pt[:, 64 * h:64 * h + 64])
                                    nc.vector.tensor_copy(
                                        va[:, 64:65], ones_sb[:, 0:1])
                                    vh_aug[h][st] = va

                # ---- attention --------------------------------------------
                at_t = [patt.tile([64, S], F32R, tag="attn", name=f"att_{b}_{hh}")
                         for hh in range(HPC)]
                for qc in range(NQC):
                    nkt = 4 * qc + 4
                    po = [pso.tile([65, 512], F32, tag="pso", name=f"po_{b}_{qc}_{hh}")
                          for hh in range(HPC)]
                    for kt in range(nkt):
                        pscore = pss.tile([128, 1024], F32, tag="pss")
                        for h in range(HPC):
                            nc.tensor.matmul(
                                pscore[:, h * 512:(h + 1) * 512],
                                kh_t[64 * h:64 * h + 64,
                                     kt * 128:(kt + 1) * 128],
                                qh_t[64 * h:64 * h + 64,
                                     qc * 512:(qc + 1) * 512],
                                start=True, stop=True)
                        ett = pet.tile([128, 1024], F32R, tag="et")
                        nc.scalar.activation(ett[:], pscore[:], EXP, scale=0.125)
                        if kt >= 4 * qc:
                            j = kt - 4 * qc
                            for h in range(HPC):
                                nc.vector.tensor_mul(
                                    ett[:, h * 512:(h + 1) * 512],
                                    ett[:, h * 512:(h + 1) * 512],
                                    masks_sb[:, j * 512:(j + 1) * 512])
                        for h in range(HPC):
                            nc.tensor.matmul(
                                po[h][:, :], vh_aug[h][kt][:],
                                ett[:, h * 512:(h + 1) * 512],
                                start=(kt == 0), stop=(kt == nkt - 1))
                    # normalize: rows 0..63 of po are unnormalized attn_outT,
                    # row 64 is the softmax denominator
                    rv = prv.tile([65, 1024], F32R, tag="rv")
                    with nc.allow_low_precision(reason="f32r softmax denom"):
                        for h in range(HPC):
                            nc.vector.reciprocal(
                                rv[64:65, h * 512:(h + 1) * 512],
                                po[h][64:65, 0:512])
                    pb = pss.tile([64, 1024], F32, tag="pss")
                    for h in range(HPC):
                        nc.tensor.matmul(
                            pb[:, h * 512:(h + 1) * 512],
                            ones_sb[64:65, 0:64],
                            rv[64:65, h * 512:(h + 1) * 512],
                            start=True, stop=True)
                    # DVE can read only one non-scalar PSUM operand; stage
                    # the broadcast through SBUF on the scalar engine
                    pb2 = prv.tile([64, 1024], F32R, tag="rvb")
                    nc.scalar.copy(pb2[:], pb[:])
                    for h in range(HPC):
                        nc.vector.tensor_mul(
                            at_t[h][:, qc * 512:(qc + 1) * 512],
                            po[h][0:64, :], pb2[0:64, h * 512:(h + 1) * 512])

                # ---- output projection (head-local raw-view reshape) ------
                for ch in range(2):
                    wts = []
                    for m in range(H):
                        wt = pwo.tile([64, 512], F32R, tag="wo",
                                      name=f"wo_{b}_{ch}_{m}")
                        nc.sync.dma_start(
                            wt[:],
                            wo[:, m * D + ch * 512:m * D + (ch + 1) * 512])
                        wts.append(wt)
                    for h in range(HPC):
                        # A_mT[e, u] = attn_outT[e, 16u + m]; out rows (u) of
                        # this head = sum_m A_m @ Wo[64m:64m+64]
                        a3 = at_t[h][:].rearrange("e (u m) -> e m u", m=H)
                        pop = psp.tile([128, 512], F32, tag="psp")
                        for m in range(H):
                            nc.tensor.matmul(
                                pop[:], a3[:, m, :], wts[m][:],
                                start=(m == 0), stop=(m == H - 1))
                        osb = posb.tile([128, 512], F32, tag="osb")
                        nc.vector.tensor_copy(osb[:], pop[:])
                        nc.sync.dma_start(
                            out[b, h, :, ch * 512:(ch + 1) * 512], osb[:])
    nc.compile()
    return nc


def _host_inputs(q, k, v, Wq, Wk, Wv, Wo):
    """Build the 8 per-core input maps (all float32 numpy)."""
    f = np.float32
    qT = np.ascontiguousarray(q.transpose(0, 2, 1)).astype(f, copy=False)
    kT = np.ascontiguousarray(k.transpose(0, 2, 1)).astype(f, copy=False)
    vT = np.ascontiguousarray(v.transpose(0, 2, 1)).astype(f, copy=False)

    def pack_w(Wh2):                      # [D, 128] -> [128, D] tile-packed
        return np.ascontiguousarray(
            Wh2.reshape(NDT, 128, 128).transpose(1, 0, 2).reshape(128, D)
        ).astype(f, copy=False)

    wo_p = np.ascontiguousarray(
        Wo.reshape(H, 64, D).transpose(1, 0, 2).reshape(64, H * D)
    ).astype(f, copy=False)

    m = np.zeros((128, 4 * 512), f)
    for j in range(4):
        m[:, j * 512:(j + 1) * 512] = (
            np.arange(128)[:, None] + 128 * j
            <= np.arange(512)[None, :]).astype(f)
    ident = np.eye(128, dtype=f)
    ones = np.ones((128, 64), f)

    in_maps = []
    for c in range(NCORES):
        W2q = np.concatenate([Wq[2 * c], Wq[2 * c + 1]], axis=1)  # [D, 128]
        W2k = np.concatenate([Wk[2 * c], Wk[2 * c + 1]], axis=1)
        W2v = np.concatenate([Wv[2 * c], Wv[2 * c + 1]], axis=1)
        in_maps.append({
            "qT": qT, "kT": kT, "vT": vT,
            "wq": pack_w(W2q), "wk": pack_w(W2k), "wv": pack_w(W2v),
            "wo": wo_p, "masks": m, "ident": ident, "ones": ones,
        })
    return in_maps


def kernel(q, k, v, Wq, Wk, Wv, Wo, _trace=False):
    from concourse.bass_utils import run_bass_kernel_spmd

    if "nc" not in _CACHE:
        _CACHE["nc"] = _build_nc()
    nc = _CACHE["nc"]

    q = np.asarray(q, np.float32)
    k = np.asarray(k, np.float32)
    v = np.asarray(v, np.float32)
    in_maps = _host_inputs(q, k, v, np.asarray(Wq, np.float32),
                           np.asarray(Wk, np.float32),
                           np.asarray(Wv, np.float32),
                           np.asarray(Wo, np.float32))
    res = run_bass_kernel_spmd(nc, in_maps, core_ids=list(range(NCORES)),
                               trace=_trace)
    OUT = np.empty((B, S, D), np.float32)
    for c in range(NCORES):
        oc = res.results[c]["out"]            # [B, HPC, SH, D]
        for b in range(B):
            for l in range(HPC):
                h = 2 * c + l
                OUT[b, h * SH:(h + 1) * SH, :] = oc[b, l]
    if _trace:
        return OUT, res
    return OUT
